# revision 1
# baseline (speedup 1.0000x reference)
"""GNN message-passing layer (DAGLayer) on 8 Trainium2 NeuronCores.

Strategy:
  - Shard destination nodes 8 ways (12544 nodes = 98 tiles of 128 per core,
    N padded 100000 -> 100352). Replicate h (as a bf16 gather table) so
    cross-partition edges need no communication.
  - Segment-sum over edges via TensorE matmuls: for each 128-dst tile,
    gather h[src] rows (dma_gather, bf16) for its edges in chunks of 128,
    build one-hot P^T on DVE (is_equal vs iota), accumulate
    segT = G^T @ P^T in PSUM ([dim, dst] layout so no transpose is needed
    downstream).
  - Edge indices are int16 for dma_gather, so h is split into 4 row-banks
    of 25088; each (tile, bank) gets a fixed number of 128-edge chunk
    slots (cap_b, normally 2) so the compiled program is identical on all
    cores (SPMD) -- only the data (indices, one-hot columns) differs.
  - Dense path per tile with host-folded weights:
      PSUM_A = h @ [Ws^T | (Wg1+Wg2@Ws)^T] + 1*[b_s|b_g+Wg2@b_s] + p*[b_n|Wg2@b_n]
      PSUM_B = seg @ [Wn^T | (Wg2@Wn)^T]
      Y      = s2 * PSUM_B + PSUM_A     (s2 = has_pred/max(counts,1))
      m, gate_pre = Y[:, :128], Y[:, 128:]
      v = h + sigmoid(gate_pre) * (m - h)
      out = relu((v - mu) * rstd)  (LayerNorm; gamma/beta folded when trivial)
"""

import math
import numpy as np

try:
    import ml_dtypes

    BF16 = np.dtype(ml_dtypes.bfloat16)
except ImportError:  # pragma: no cover
    BF16 = None

N = 100000
E = 600000
D = 128
N_CORES = 8
NPC = 12544            # nodes per core
TPC = NPC // 128       # 98 tiles per core
N_PAD = NPC * N_CORES  # 100352
N_BANKS = 4
BANK = N_PAD // N_BANKS  # 25088
TG = 14                # tiles per gather group
N_GROUPS = TPC // TG   # 7
LN_EPS = 1e-5


def _build_schedule(edge_src, edge_dst):
    """Chunk/slot schedule. Returns per-core idx16 + d_cols arrays and the
    (core-independent) slot capacities."""
    edge_src = np.asarray(edge_src, np.int64)
    edge_dst = np.asarray(edge_dst, np.int64)

    tile_g = edge_dst // 128          # global dst tile 0..783
    bank = edge_src // BANK           # 0..3
    key = tile_g * N_BANKS + bank
    order = np.argsort(key, kind="stable")
    src_s = edge_src[order]
    dst_s = edge_dst[order]
    key_s = key[order]

    n_tiles_g = N_PAD // 128          # 784
    cnt = np.bincount(key_s, minlength=n_tiles_g * N_BANKS).reshape(
        n_tiles_g, N_BANKS
    )
    starts = np.zeros((n_tiles_g, N_BANKS), np.int64)
    starts.reshape(-1)[1:] = np.cumsum(cnt.reshape(-1))[:-1]

    caps = [max(1, int(np.ceil(cnt[:, b].max() / 128.0))) for b in range(N_BANKS)]
    S_t = sum(caps)                   # slots per tile
    cap_off = np.cumsum([0] + caps)   # slot offset of bank b within a tile

    # per-core arrays
    idx16 = np.zeros((N_CORES, N_GROUPS, N_BANKS), object)
    d_cols = np.full((N_CORES, 128, TPC * S_t), -1.0, np.float32)

    for c in range(N_CORES):
        for g in range(N_GROUPS):
            for b in range(N_BANKS):
                lst = np.zeros(TG * caps[b] * 128, np.int16)
                for ti in range(TG):
                    t_local = g * TG + ti
                    t_glob = c * TPC + t_local
                    n_e = int(cnt[t_glob, b])
                    s0 = int(starts[t_glob, b])
                    srcs = src_s[s0 : s0 + n_e] - b * BANK
                    dsts = dst_s[s0 : s0 + n_e] - t_glob * 128
                    base = ti * caps[b] * 128
                    lst[base : base + n_e] = srcs.astype(np.int16)
                    # d_cols slot columns for (t_local, bank b, chunk j)
                    for j in range(caps[b]):
                        col = t_local * S_t + cap_off[b] + j
                        seg = dsts[j * 128 : (j + 1) * 128]
                        d_cols[c, : len(seg), col] = seg
                idx16[c, g, b] = lst
    return idx16, d_cols, caps, S_t, cap_off


def _wrap_idx16(flat):
    """dma_gather index layout: idx i -> [i % 16, i // 16], replicated to
    all 128 partitions."""
    n = len(flat)
    w = np.zeros((128, n // 16), np.int16)
    w16 = flat.reshape(n // 16, 16).T  # [16, n/16]
    w[:16] = w16
    w[16:] = np.tile(w16, (7, 1))
    return w


def _fold_weights(W_self, b_self, W_neigh, b_neigh, W_gate, b_gate):
    Wg1 = W_gate[:, :D]
    Wg2 = W_gate[:, D:]
    rhs_A = np.concatenate([W_self.T, (Wg1 + Wg2 @ W_self).T], 1)  # [128,256]
    rhs_B = np.concatenate([W_neigh.T, (Wg2 @ W_neigh).T], 1)      # [128,256]
    bias_A = np.concatenate([b_self, b_gate + Wg2 @ b_self])[None]  # [1,256]
    bias_Bp = np.concatenate([b_neigh, Wg2 @ b_neigh])[None]        # [1,256]
    return (
        np.ascontiguousarray(rhs_A, np.float32),
        np.ascontiguousarray(rhs_B, np.float32),
        np.ascontiguousarray(bias_A, np.float32),
        np.ascontiguousarray(bias_Bp, np.float32),
    )


def _prep(h, edge_src, edge_dst, W_self, b_self, W_neigh, b_neigh, W_gate,
          b_gate, ln_gamma, ln_beta):
    h = np.asarray(h, np.float32)
    h_pad = np.zeros((N_PAD, D), np.float32)
    h_pad[:N] = h
    h_bf16 = h_pad.astype(BF16)

    idx16, d_cols, caps, S_t, cap_off = _build_schedule(edge_src, edge_dst)

    counts = np.bincount(np.asarray(edge_dst, np.int64), minlength=N_PAD)
    p = (counts > 0).astype(np.float32)
    s2 = p / np.maximum(counts, 1).astype(np.float32)

    rhs_A, rhs_B, bias_A, bias_Bp = _fold_weights(
        np.asarray(W_self, np.float32), np.asarray(b_self, np.float32),
        np.asarray(W_neigh, np.float32), np.asarray(b_neigh, np.float32),
        np.asarray(W_gate, np.float32), np.asarray(b_gate, np.float32))

    trivial_ln = (np.allclose(ln_gamma, 1.0) and np.allclose(ln_beta, 0.0))

    per_core = []
    for c in range(N_CORES):
        rows = slice(c * NPC, (c + 1) * NPC)
        idx_segs = []
        for g in range(N_GROUPS):
            for b in range(N_BANKS):
                idx_segs.append(_wrap_idx16(idx16[c, g, b]))
        idx_all = np.concatenate(idx_segs, axis=1)  # [128, sum(n/16)]
        per_core.append(dict(
            h_self=np.ascontiguousarray(h_pad[rows]),
            hT_self=np.ascontiguousarray(h_pad[rows].T),
            h_gather=h_bf16,
            idx_all=idx_all,
            d_cols=np.ascontiguousarray(d_cols[c]),
            s2_cols=np.ascontiguousarray(
                s2[rows].reshape(TPC, 128).T),          # [128, 98]
            p_row=np.ascontiguousarray(p[rows][None]),  # [1, 12544]
            rhs_A=rhs_A, rhs_B=rhs_B, bias_A=bias_A, bias_Bp=bias_Bp,
            iota=np.ascontiguousarray(
                np.broadcast_to(np.arange(128, dtype=np.float32), (128, 128))
            ).astype(BF16),
            ones_row=np.ones((1, 128), np.float32),
        ))
    meta = dict(caps=caps, S_t=S_t, cap_off=cap_off, trivial_ln=trivial_ln,
                ln_gamma=np.asarray(ln_gamma, np.float32),
                ln_beta=np.asarray(ln_beta, np.float32))
    return per_core, meta


def _sim_core(pc, meta):
    """Numpy simulation of the device program for one core (bf16 where the
    device uses bf16). Returns [NPC, 128] float32."""
    caps, S_t, cap_off = meta["caps"], meta["S_t"], meta["cap_off"]
    h_gather = np.asarray(pc["h_gather"], BF16).astype(np.float32)
    d_cols = np.asarray(pc["d_cols"], np.float32)
    out = np.zeros((NPC, D), np.float32)

    # rebuild flat idx lists from the wrapped layout
    seg_len = [TG * caps[b] * 128 for b in range(N_BANKS)]
    idx_all = pc["idx_all"]
    col = 0
    flat_idx = {}
    for g in range(N_GROUPS):
        for b in range(N_BANKS):
            n = seg_len[b]
            w = idx_all[:16, col : col + n // 16]
            flat_idx[(g, b)] = w.T.reshape(-1)
            col += n // 16

    for g in range(N_GROUPS):
        # G slots, bank-major: [slots, 128, 128]
        G = np.zeros((TG * S_t, 128, D), np.float32)
        for b in range(N_BANKS):
            fl = flat_idx[(g, b)].astype(np.int64) + b * BANK
            rows = h_gather[fl].reshape(TG, caps[b], 128, D)
            for ti in range(TG):
                for j in range(caps[b]):
                    slot = b * TG * caps[b] + ti * caps[b] + j
                    G[slot] = rows[ti, j]
        for ti in range(TG):
            t = g * TG + ti
            psum_S = np.zeros((D, 128), np.float32)  # [dim, dst]
            for b in range(N_BANKS):
                for j in range(caps[b]):
                    slot = b * TG * caps[b] + ti * caps[b] + j
                    dcol = d_cols[:, t * S_t + cap_off[b] + j]  # [128]
                    PT = (dcol[:, None] == np.arange(128)[None, :]).astype(
                        np.float32)  # [e, dst]
                    Gc = G[slot]  # [e, dim]
                    psum_S += Gc.T @ PT
            segT = psum_S  # [dim, dst]
            hT = pc["hT_self"][:, t * 128 : (t + 1) * 128]  # [dim, node]
            p_r = pc["p_row"][0, t * 128 : (t + 1) * 128]   # [node]
            psum_A = (hT.T @ pc["rhs_A"] + np.ones((128, 1)) @ pc["bias_A"]
                      + p_r[:, None] @ pc["bias_Bp"])
            psum_B = segT.T @ pc["rhs_B"]
            s2c = pc["s2_cols"][:, t]  # [128]
            Y = s2c[:, None] * psum_B + psum_A
            m, gp = Y[:, :D], Y[:, D:]
            hrow = pc["h_self"][t * 128 : (t + 1) * 128]
            gate = 1.0 / (1.0 + np.exp(-gp))
            v = hrow + gate * (m - hrow)
            mu = v.mean(1, keepdims=True)
            var = (v * v).mean(1, keepdims=True) - mu * mu
            rstd = 1.0 / np.sqrt(var + LN_EPS)
            z = (v - mu) * rstd
            if not meta["trivial_ln"]:
                z = z * meta["ln_gamma"][None] + meta["ln_beta"][None]
            out[t * 128 : (t + 1) * 128] = np.maximum(z, 0.0)
    return out


def kernel_numpy_sim(**inputs):
    per_core, meta = _prep(**inputs)
    outs = [_sim_core(pc, meta) for pc in per_core]
    return np.concatenate(outs, 0)[:N]


# ---------------------------------------------------------------------------
# Bass device kernel
# ---------------------------------------------------------------------------

_BASS_CACHE = {}


def _build_bass(caps, S_t, cap_off, trivial_ln, idxcols, gamma=None, beta=None):
    import concourse.bacc as bacc
    import concourse.bass as bass
    import concourse.tile as tile
    from concourse import mybir

    f32 = mybir.dt.float32
    bf16 = mybir.dt.bfloat16
    i16 = mybir.dt.int16
    Alu = mybir.AluOpType
    Act = mybir.ActivationFunctionType

    nc = bacc.Bacc("TRN2", target_bir_lowering=False, debug=False,
                   num_devices=N_CORES)

    h_self = nc.dram_tensor("h_self", [NPC, D], f32, kind="ExternalInput")
    hT_self = nc.dram_tensor("hT_self", [D, NPC], f32, kind="ExternalInput")
    h_gather = nc.dram_tensor("h_gather", [N_PAD, D], bf16, kind="ExternalInput")
    idx_all = nc.dram_tensor("idx_all", [128, idxcols], i16, kind="ExternalInput")
    d_cols_d = nc.dram_tensor("d_cols", [128, TPC * S_t], f32, kind="ExternalInput")
    s2_cols_d = nc.dram_tensor("s2_cols", [128, TPC], f32, kind="ExternalInput")
    p_row_d = nc.dram_tensor("p_row", [1, NPC], f32, kind="ExternalInput")
    rhs_A_d = nc.dram_tensor("rhs_A", [D, 2 * D], f32, kind="ExternalInput")
    rhs_B_d = nc.dram_tensor("rhs_B", [D, 2 * D], f32, kind="ExternalInput")
    bias_A_d = nc.dram_tensor("bias_A", [1, 2 * D], f32, kind="ExternalInput")
    bias_Bp_d = nc.dram_tensor("bias_Bp", [1, 2 * D], f32, kind="ExternalInput")
    iota_d = nc.dram_tensor("iota", [128, 128], bf16, kind="ExternalInput")
    ones_d = nc.dram_tensor("ones_row", [1, 128], f32, kind="ExternalInput")
    out_d = nc.dram_tensor("out", [NPC, D], f32, kind="ExternalOutput")

    b_off = [TG * sum(caps[:b]) for b in range(N_BANKS)]  # G slot region start
    seg_w = [TG * caps[b] * 128 // 16 for b in range(N_BANKS)]  # idx cols/(g,b)

    with tile.TileContext(nc) as tc:
        with (
            tc.tile_pool(name="consts", bufs=1) as cpool,
            tc.tile_pool(name="gbuf", bufs=2) as gpool,
            tc.tile_pool(name="ptbuf", bufs=2) as ptpool,
            tc.tile_pool(name="work", bufs=3) as wpool,
            tc.tile_pool(name="small", bufs=4) as spool,
            tc.tile_pool(name="psum", bufs=2, space="PSUM") as psum,
        ):
            # --- constants, loaded once ---
            idx_sb = cpool.tile([128, idxcols], i16)
            nc.sync.dma_start(out=idx_sb[:], in_=idx_all[:])
            d_cols_sb = cpool.tile([128, TPC * S_t], f32)
            nc.sync.dma_start(out=d_cols_sb[:], in_=d_cols_d[:])
            s2_sb = cpool.tile([128, TPC], f32)
            nc.sync.dma_start(out=s2_sb[:], in_=s2_cols_d[:])
            p_sb = cpool.tile([1, NPC], f32)
            nc.sync.dma_start(out=p_sb[:], in_=p_row_d[:])
            rhsA_sb = cpool.tile([D, 2 * D], f32)
            nc.sync.dma_start(out=rhsA_sb[:], in_=rhs_A_d[:])
            rhsB_sb = cpool.tile([D, 2 * D], f32)
            nc.sync.dma_start(out=rhsB_sb[:], in_=rhs_B_d[:])
            biasA_sb = cpool.tile([1, 2 * D], f32)
            nc.sync.dma_start(out=biasA_sb[:], in_=bias_A_d[:])
            biasBp_sb = cpool.tile([1, 2 * D], f32)
            nc.sync.dma_start(out=biasBp_sb[:], in_=bias_Bp_d[:])
            iota_sb = cpool.tile([128, 128], bf16)
            nc.sync.dma_start(out=iota_sb[:], in_=iota_d[:])
            ones_sb = cpool.tile([1, 128], f32)
            nc.sync.dma_start(out=ones_sb[:], in_=ones_d[:])
            eps_sb = cpool.tile([128, 1], f32)
            nc.vector.memset(eps_sb[:], LN_EPS)
            if not trivial_ln:
                gamma_sb = cpool.tile([128, D], f32)
                nc.gpsimd.dma_start(
                    out=gamma_sb[:],
                    in_=bass.AP(tensor=nc.dram_tensor(
                        "ln_gamma", [1, D], f32, kind="ExternalInput"),
                        offset=0, ap=[[0, 128], [1, D]]))
                beta_sb = cpool.tile([128, D], f32)
                nc.gpsimd.dma_start(
                    out=beta_sb[:],
                    in_=bass.AP(tensor=nc.dram_tensor(
                        "ln_beta", [1, D], f32, kind="ExternalInput"),
                        offset=0, ap=[[0, 128], [1, D]]))

            idx_off = [0]
            for g in range(N_GROUPS):
                for b in range(N_BANKS):
                    idx_off.append(idx_off[-1] + seg_w[b])

            for g in range(N_GROUPS):
                G = gpool.tile([128, TG * S_t, 128], bf16, tag="G")
                for b in range(N_BANKS):
                    o = idx_off[g * N_BANKS + b]
                    nidx = TG * caps[b] * 128
                    nc.gpsimd.dma_gather(
                        out_ap=G[:, b_off[b] : b_off[b] + TG * caps[b], :],
                        in_ap=h_gather[b * BANK :, :],
                        idxs_ap=idx_sb[:, o : o + seg_w[b]],
                        num_idxs=nidx,
                        num_idxs_reg=nidx,
                        elem_size=D,
                        single_packet=False,
                    )
                for ti in range(TG):
                    t = g * TG + ti
                    # --- one-hot P^T for the 8 slots ---
                    PT = ptpool.tile([128, S_t, 128], bf16, tag="PT")
                    for k in range(S_t):
                        nc.vector.tensor_scalar(
                            out=PT[:, k, :], in0=iota_sb[:],
                            scalar1=d_cols_sb[:, t * S_t + k : t * S_t + k + 1],
                            scalar2=None, op0=Alu.is_equal)
                    # --- segment sum: segT [dim, dst] ---
                    ps_S = psum.tile([128, 128], f32, tag="psS")
                    nmm = 0
                    for b in range(N_BANKS):
                        for j in range(caps[b]):
                            slot = b_off[b] + ti * caps[b] + j
                            k = cap_off[b] + j
                            nc.tensor.matmul(
                                ps_S[:], lhsT=G[:, slot, :], rhs=PT[:, k, :],
                                start=(nmm == 0), stop=(nmm == S_t - 1))
                            nmm += 1
                    segT = wpool.tile([128, 128], f32, tag="segT")
                    nc.scalar.copy(out=segT[:], in_=ps_S[:])

                    # --- dense matmuls ---
                    hT_t = wpool.tile([128, 128], f32, tag="hT")
                    nc.sync.dma_start(out=hT_t[:], in_=hT_self[:, t * 128 : (t + 1) * 128])
                    h_t = wpool.tile([128, 128], f32, tag="h")
                    nc.sync.dma_start(out=h_t[:], in_=h_self[t * 128 : (t + 1) * 128, :])

                    ps_A = psum.tile([128, 2 * D], f32, tag="psA")
                    nc.tensor.matmul(ps_A[:], lhsT=hT_t[:], rhs=rhsA_sb[:],
                                     start=True, stop=False)
                    nc.tensor.matmul(ps_A[:], lhsT=ones_sb[:], rhs=biasA_sb[:],
                                     start=False, stop=False)
                    nc.tensor.matmul(ps_A[:], lhsT=p_sb[:, t * 128 : (t + 1) * 128],
                                     rhs=biasBp_sb[:], start=False, stop=True)
                    ps_B = psum.tile([128, 2 * D], f32, tag="psB")
                    nc.tensor.matmul(ps_B[:], lhsT=segT[:], rhs=rhsB_sb[:],
                                     start=True, stop=True)

                    # --- elementwise ---
                    a_sb = wpool.tile([128, 2 * D], f32, tag="a_sb")
                    nc.scalar.copy(out=a_sb[:], in_=ps_A[:])
                    Y = wpool.tile([128, 2 * D], f32, tag="Y")
                    nc.vector.scalar_tensor_tensor(
                        out=Y[:], in0=ps_B[:], scalar=s2_sb[:, t : t + 1],
                        in1=a_sb[:], op0=Alu.mult, op1=Alu.add)
                    g_sb = wpool.tile([128, D], bf16, tag="g_sb")
                    nc.scalar.activation(out=g_sb[:], in_=Y[:, D:], func=Act.Sigmoid)
                    t1 = wpool.tile([128, D], f32, tag="t1")
                    nc.gpsimd.tensor_tensor(out=t1[:], in0=Y[:, :D], in1=h_t[:],
                                            op=Alu.subtract)
                    t2 = wpool.tile([128, D], f32, tag="t2")
                    nc.vector.tensor_tensor(out=t2[:], in0=g_sb[:], in1=t1[:],
                                            op=Alu.mult)
                    v = wpool.tile([128, D], f32, tag="v")
                    sumv = spool.tile([128, 1], f32, tag="sumv")
                    nc.vector.scalar_tensor_tensor(
                        out=v[:], in0=t2[:], scalar=0.0, in1=h_t[:],
                        op0=Alu.add, op1=Alu.add, accum_out=sumv[:])
                    vsq = wpool.tile([128, D], f32, tag="vsq")
                    sv2 = spool.tile([128, 1], f32, tag="sv2")
                    nc.scalar.activation(out=vsq[:], in_=v[:], func=Act.Square,
                                         accum_out=sv2[:])
                    meansq = spool.tile([128, 1], f32, tag="meansq")
                    nc.vector.tensor_scalar(out=meansq[:], in0=sv2[:],
                                            scalar1=1.0 / D, scalar2=None,
                                            op0=Alu.mult)
                    mu = spool.tile([128, 1], f32, tag="mu")
                    nc.vector.tensor_scalar(out=mu[:], in0=sumv[:],
                                            scalar1=1.0 / D, scalar2=None,
                                            op0=Alu.mult)
                    negvar = spool.tile([128, 1], f32, tag="negvar")
                    nc.vector.scalar_tensor_tensor(
                        out=negvar[:], in0=mu[:], scalar=mu[:], in1=meansq[:],
                        op0=Alu.mult, op1=Alu.subtract)
                    std = spool.tile([128, 1], f32, tag="std")
                    nc.scalar.activation(out=std[:], in_=negvar[:], func=Act.Sqrt,
                                         bias=eps_sb[:], scale=-1.0)
                    rstd = spool.tile([128, 1], f32, tag="rstd")
                    nc.vector.reciprocal(out=rstd[:], in_=std[:])
                    bias_ln = spool.tile([128, 1], f32, tag="bias_ln")
                    nc.vector.scalar_tensor_tensor(
                        out=bias_ln[:], in0=mu[:], scalar=-1.0, in1=rstd[:],
                        op0=Alu.mult, op1=Alu.mult)
                    out_t = wpool.tile([128, D], f32, tag="out_t")
                    if trivial_ln:
                        nc.scalar.activation(out=out_t[:], in_=v[:], func=Act.Relu,
                                             bias=bias_ln[:], scale=rstd[:])
                    else:
                        z = wpool.tile([128, D], f32, tag="z")
                        nc.scalar.activation(out=z[:], in_=v[:], func=Act.Identity,
                                             bias=bias_ln[:], scale=rstd[:])
                        nc.vector.tensor_tensor(out=z[:], in0=z[:], in1=gamma_sb[:],
                                                op=Alu.mult)
                        nc.vector.tensor_tensor(out=z[:], in0=z[:], in1=beta_sb[:],
                                                op=Alu.add)
                        nc.scalar.activation(out=out_t[:], in_=z[:], func=Act.Relu)
                    nc.sync.dma_start(out=out_d[t * 128 : (t + 1) * 128, :],
                                      in_=out_t[:])
    nc.compile()
    return nc


def kernel(**inputs):
    from concourse.bass_utils import run_bass_kernel_spmd

    per_core, meta = _prep(**{k: np.asarray(v) for k, v in inputs.items()})
    idxcols = per_core[0]["idx_all"].shape[1]
    key = (tuple(meta["caps"]), meta["trivial_ln"], idxcols)
    if key not in _BASS_CACHE:
        _BASS_CACHE[key] = _build_bass(
            meta["caps"], meta["S_t"], meta["cap_off"], meta["trivial_ln"],
            idxcols)
    nc = _BASS_CACHE[key]

    in_maps = []
    for pc in per_core:
        m = dict(pc)
        if not meta["trivial_ln"]:
            m["ln_gamma"] = meta["ln_gamma"][None]
            m["ln_beta"] = meta["ln_beta"][None]
        in_maps.append(m)
    res = run_bass_kernel_spmd(nc, in_maps, core_ids=list(range(N_CORES)))
    outs = [res.results[c]["out"] for c in range(N_CORES)]
    return np.concatenate(outs, 0)[:N].astype(np.float32)



# revision 5
# speedup vs baseline: 2.5101x; 2.5101x over previous
"""GNN message-passing layer (DAGLayer) on 8 Trainium2 NeuronCores.

Strategy (v2):
  - Shard destination nodes 8 ways (12544 nodes = 98 tiles of 128 per core,
    N padded 100000 -> 100352). Replicate h (as a bf16 gather table) so
    cross-partition edges need no communication.
  - Segment-sum over edges via TensorE matmuls: for each 128-dst tile,
    gather h[src] rows (dma_gather, bf16) for its edges in chunks of 128,
    multiply with host-prebuilt one-hot planes PT (scaled by 1/count so the
    matmul yields the neighbor MEAN directly), accumulate segT = G^T @ PT
    in PSUM ([dim, dst] layout).
  - The GpSimd engine queue carries ONLY the dma_gathers so descriptor
    generation (the bottleneck) overlaps all compute.
  - Dense path per tile with host-folded weights, all bf16, accumulated in
    one PSUM bank:
      PSUM_Y = h @ [Ws^T | (Wg1+Wg2@Ws)^T] + [1;p]^T @ [[b_s|b_g+Wg2@b_s],
               [b_n|Wg2@b_n]] + seg_mean @ [Wn^T | (Wg2@Wn)^T]
      m, gate_pre = Y[:, :128], Y[:, 128:]
      v = h + sigmoid(gate_pre) * (m - h)
      out = relu((v - mu) * rstd)   (LayerNorm, batched per group of 7
      tiles on DVE with a single Rsqrt activation)
"""

import math
import numpy as np

try:
    import ml_dtypes

    BF16 = np.dtype(ml_dtypes.bfloat16)
except ImportError:  # pragma: no cover
    BF16 = None

N = 100000
E = 600000
D = 128
N_CORES = 8
NPC = 12544            # nodes per core
TPC = NPC // 128       # 98 tiles per core
N_PAD = NPC * N_CORES  # 100352
N_BANKS = 4
BANK = N_PAD // N_BANKS  # 25088
TG = 7                 # tiles per gather group
N_GROUPS = TPC // TG   # 14
LN_EPS = 1e-5


def _build_schedule(edge_src, edge_dst):
    """Chunk/slot schedule. Returns per-core idx16 arrays, per-core PT
    one-hot planes (scaled by 1/count), and the slot capacities."""
    edge_src = np.asarray(edge_src, np.int64)
    edge_dst = np.asarray(edge_dst, np.int64)

    tile_g = edge_dst // 128          # global dst tile 0..783
    bank = edge_src // BANK           # 0..3
    key = tile_g * N_BANKS + bank
    order = np.argsort(key, kind="stable")
    src_s = edge_src[order]
    dst_s = edge_dst[order]

    n_tiles_g = N_PAD // 128          # 784
    cnt = np.bincount(key[order], minlength=n_tiles_g * N_BANKS).reshape(
        n_tiles_g, N_BANKS
    )
    starts = np.zeros((n_tiles_g, N_BANKS), np.int64)
    starts.reshape(-1)[1:] = np.cumsum(cnt.reshape(-1))[:-1]

    caps = [max(1, int(np.ceil(cnt[:, b].max() / 128.0))) for b in range(N_BANKS)]
    S_t = sum(caps)                   # slots per tile
    cap_off = np.cumsum([0] + caps)   # slot offset of bank b within a tile

    counts_n = np.bincount(edge_dst, minlength=N_PAD).astype(np.float64)
    inv_cnt = np.zeros(N_PAD, np.float32)
    nz = counts_n > 0
    inv_cnt[nz] = (1.0 / counts_n[nz]).astype(np.float32)

    idx16 = np.zeros((N_CORES, N_GROUPS, N_BANKS), object)
    pt_all = np.zeros((N_CORES, 128, TPC * S_t * 128), np.float32)

    for c in range(N_CORES):
        pt_c = pt_all[c]
        for g in range(N_GROUPS):
            for b in range(N_BANKS):
                lst = np.zeros(TG * caps[b] * 128, np.int16)
                for ti in range(TG):
                    t_local = g * TG + ti
                    t_glob = c * TPC + t_local
                    n_e = int(cnt[t_glob, b])
                    s0 = int(starts[t_glob, b])
                    srcs = src_s[s0 : s0 + n_e] - b * BANK
                    dsts = dst_s[s0 : s0 + n_e]
                    base = ti * caps[b] * 128
                    lst[base : base + n_e] = srcs.astype(np.int16)
                    pos = np.arange(n_e)
                    rows = pos % 128
                    js = pos // 128
                    slot = t_local * S_t + cap_off[b] + js
                    cols = slot * 128 + (dsts - t_glob * 128)
                    pt_c[rows, cols] = inv_cnt[dsts]
                idx16[c, g, b] = lst
    return idx16, pt_all, caps, S_t, cap_off


def _wrap_idx16(flat):
    """dma_gather index layout: idx i -> [i % 16, i // 16], replicated to
    all 128 partitions."""
    n = len(flat)
    w = np.zeros((128, n // 16), np.int16)
    w16 = flat.reshape(n // 16, 16).T  # [16, n/16]
    w[:16] = w16
    w[16:] = np.tile(w16, (7, 1))
    return w


def _fold_weights(W_self, b_self, W_neigh, b_neigh, W_gate, b_gate):
    Wg1 = W_gate[:, :D]
    Wg2 = W_gate[:, D:]
    rhs_A = np.concatenate([W_self.T, (Wg1 + Wg2 @ W_self).T], 1)  # [128,256]
    rhs_B = np.concatenate([W_neigh.T, (Wg2 @ W_neigh).T], 1)      # [128,256]
    bias_A = np.concatenate([b_self, b_gate + Wg2 @ b_self])[None]  # [1,256]
    bias_Bp = np.concatenate([b_neigh, Wg2 @ b_neigh])[None]        # [1,256]
    return (
        np.ascontiguousarray(rhs_A, np.float32),
        np.ascontiguousarray(rhs_B, np.float32),
        np.ascontiguousarray(bias_A, np.float32),
        np.ascontiguousarray(bias_Bp, np.float32),
    )


def _prep(h, edge_src, edge_dst, W_self, b_self, W_neigh, b_neigh, W_gate,
          b_gate, ln_gamma, ln_beta):
    h = np.asarray(h, np.float32)
    h_pad = np.zeros((N_PAD, D), np.float32)
    h_pad[:N] = h
    h_bf16 = h_pad.astype(BF16)

    idx16, pt_all, caps, S_t, cap_off = _build_schedule(edge_src, edge_dst)

    counts = np.bincount(np.asarray(edge_dst, np.int64), minlength=N_PAD)
    p = (counts > 0).astype(np.float32)

    rhs_A, rhs_B, bias_A, bias_Bp = _fold_weights(
        np.asarray(W_self, np.float32), np.asarray(b_self, np.float32),
        np.asarray(W_neigh, np.float32), np.asarray(b_neigh, np.float32),
        np.asarray(W_gate, np.float32), np.asarray(b_gate, np.float32))
    bias_AB = np.concatenate([bias_A, bias_Bp], 0)  # [2, 256]

    trivial_ln = (np.allclose(ln_gamma, 1.0) and np.allclose(ln_beta, 0.0))

    per_core = []
    for c in range(N_CORES):
        rows = slice(c * NPC, (c + 1) * NPC)
        idx_segs = []
        for g in range(N_GROUPS):
            for b in range(N_BANKS):
                idx_segs.append(_wrap_idx16(idx16[c, g, b]))
        idx_all = np.concatenate(idx_segs, axis=1)  # [128, sum(n/16)]
        ones_p = np.stack([np.ones(NPC, np.float32), p[rows]], 0)  # [2, NPC]
        per_core.append(dict(
            h_self=np.ascontiguousarray(h_bf16[rows]),
            hT_self=np.ascontiguousarray(h_bf16[rows].T),
            h_gather=h_bf16,
            idx_all=idx_all,
            pt_all=np.ascontiguousarray(pt_all[c].astype(BF16)),
            ones_p=np.ascontiguousarray(ones_p.astype(BF16)),
            rhs_A=np.ascontiguousarray(rhs_A.astype(BF16)),
            rhs_B=np.ascontiguousarray(rhs_B.astype(BF16)),
            bias_AB=np.ascontiguousarray(bias_AB.astype(BF16)),
        ))
    meta = dict(caps=caps, S_t=S_t, cap_off=cap_off, trivial_ln=trivial_ln,
                ln_gamma=np.asarray(ln_gamma, np.float32),
                ln_beta=np.asarray(ln_beta, np.float32))
    return per_core, meta


# ---------------------------------------------------------------------------
# Bass device kernel
# ---------------------------------------------------------------------------

_BASS_CACHE = {}


def _build_bass(caps, S_t, cap_off, trivial_ln, idxcols):
    import concourse.bacc as bacc
    import concourse.bass as bass
    import concourse.tile as tile
    from concourse import mybir

    f32 = mybir.dt.float32
    bf16 = mybir.dt.bfloat16
    i16 = mybir.dt.int16
    Alu = mybir.AluOpType
    Act = mybir.ActivationFunctionType
    Axis = mybir.AxisListType

    nc = bacc.Bacc("TRN2", target_bir_lowering=False, debug=False,
                   num_devices=N_CORES)

    h_self = nc.dram_tensor("h_self", [NPC, D], bf16, kind="ExternalInput")
    hT_self = nc.dram_tensor("hT_self", [D, NPC], bf16, kind="ExternalInput")
    h_gather = nc.dram_tensor("h_gather", [N_PAD, D], bf16, kind="ExternalInput")
    idx_all = nc.dram_tensor("idx_all", [128, idxcols], i16, kind="ExternalInput")
    pt_all_d = nc.dram_tensor("pt_all", [128, TPC * S_t * 128], bf16,
                              kind="ExternalInput")
    ones_p_d = nc.dram_tensor("ones_p", [2, NPC], bf16, kind="ExternalInput")
    rhs_A_d = nc.dram_tensor("rhs_A", [D, 2 * D], bf16, kind="ExternalInput")
    rhs_B_d = nc.dram_tensor("rhs_B", [D, 2 * D], bf16, kind="ExternalInput")
    bias_AB_d = nc.dram_tensor("bias_AB", [2, 2 * D], bf16, kind="ExternalInput")
    out_d = nc.dram_tensor("out", [NPC, D], f32, kind="ExternalOutput")

    b_off = [TG * sum(caps[:b]) for b in range(N_BANKS)]  # G slot region start
    seg_w = [TG * caps[b] * 128 // 16 for b in range(N_BANKS)]  # idx cols/(g,b)

    with tile.TileContext(nc) as tc:
        with (
            tc.tile_pool(name="consts", bufs=1) as cpool,
            tc.tile_pool(name="gbuf", bufs=2) as gpool,
            tc.tile_pool(name="ptbuf", bufs=2) as ptpool,
            tc.tile_pool(name="hbuf", bufs=2) as hpool,
            tc.tile_pool(name="obuf", bufs=2) as opool,
            tc.tile_pool(name="slab", bufs=2) as lpool,
            tc.tile_pool(name="small", bufs=2) as spool,
            tc.tile_pool(name="seg", bufs=3) as segpool,
            tc.tile_pool(name="psS", bufs=2, space="PSUM") as psSpool,
            tc.tile_pool(name="psY", bufs=3, space="PSUM") as psYpool,
        ):
            # --- constants, loaded once ---
            idx_sb = cpool.tile([128, idxcols], i16)
            nc.sync.dma_start(out=idx_sb[:], in_=idx_all[:])
            hT_sb = cpool.tile([D, NPC], bf16)
            nc.sync.dma_start(out=hT_sb[:], in_=hT_self[:])
            onesp_sb = cpool.tile([2, NPC], bf16)
            nc.sync.dma_start(out=onesp_sb[:], in_=ones_p_d[:])
            rhsA_sb = cpool.tile([D, 2 * D], bf16)
            nc.sync.dma_start(out=rhsA_sb[:], in_=rhs_A_d[:])
            rhsB_sb = cpool.tile([D, 2 * D], bf16)
            nc.sync.dma_start(out=rhsB_sb[:], in_=rhs_B_d[:])
            biasAB_sb = cpool.tile([2, 2 * D], bf16)
            nc.sync.dma_start(out=biasAB_sb[:], in_=bias_AB_d[:])
            eps_sb = cpool.tile([128, 1], f32)
            nc.vector.memset(eps_sb[:], LN_EPS)
            if not trivial_ln:
                gamma_sb = cpool.tile([128, D], f32)
                nc.gpsimd.dma_start(
                    out=gamma_sb[:],
                    in_=bass.AP(tensor=nc.dram_tensor(
                        "ln_gamma", [1, D], f32, kind="ExternalInput"),
                        offset=0, ap=[[0, 128], [1, D]]))
                beta_sb = cpool.tile([128, D], f32)
                nc.gpsimd.dma_start(
                    out=beta_sb[:],
                    in_=bass.AP(tensor=nc.dram_tensor(
                        "ln_beta", [1, D], f32, kind="ExternalInput"),
                        offset=0, ap=[[0, 128], [1, D]]))

            idx_off = [0]
            for g in range(N_GROUPS):
                for b in range(N_BANKS):
                    idx_off.append(idx_off[-1] + seg_w[b])

            for g in range(N_GROUPS):
                # ---- group loads ----
                G = gpool.tile([128, TG * S_t, 128], bf16, tag="G")
                for b in range(N_BANKS):
                    o = idx_off[g * N_BANKS + b]
                    nidx = TG * caps[b] * 128
                    nc.gpsimd.dma_gather(
                        out_ap=G[:, b_off[b] : b_off[b] + TG * caps[b], :],
                        in_ap=h_gather[b * BANK :, :],
                        idxs_ap=idx_sb[:, o : o + seg_w[b]],
                        num_idxs=nidx,
                        num_idxs_reg=nidx,
                        elem_size=D,
                        single_packet=False,
                    )
                PTt = ptpool.tile([128, TG * S_t * 128], bf16, tag="PT")
                nc.sync.dma_start(
                    out=PTt[:],
                    in_=pt_all_d[:, g * TG * S_t * 128 : (g + 1) * TG * S_t * 128])
                h_g = hpool.tile([128, TG, 128], bf16, tag="hg")
                nc.sync.dma_start(
                    out=h_g[:],
                    in_=bass.AP(tensor=h_self, offset=g * TG * 128 * D,
                                ap=[[D, 128], [128 * D, TG], [1, D]]))

                out_stage = opool.tile([128, TG, 128], f32, tag="ostage")
                t1_slab = lpool.tile([128, TG, 128], f32, tag="t1")
                g_slab = lpool.tile([128, TG, 128], bf16, tag="gs")
                t2_slab = lpool.tile([128, TG, 128], f32, tag="t2")
                v_slab = lpool.tile([128, TG, 128], f32, tag="v")
                vsq_slab = lpool.tile([128, TG, 128], f32, tag="vsq")

                # ---- per-tile matmuls + PSUM-adjacent ops ----
                for ti in range(TG):
                    t = g * TG + ti
                    ps_S = psSpool.tile([128, 128], f32, tag="psS")
                    nmm = 0
                    for b in range(N_BANKS):
                        for j in range(caps[b]):
                            slot = b_off[b] + ti * caps[b] + j
                            k = cap_off[b] + j
                            col = (ti * S_t + k) * 128
                            nc.tensor.matmul(
                                ps_S[:], lhsT=G[:, slot, :],
                                rhs=PTt[:, col : col + 128],
                                start=(nmm == 0), stop=(nmm == S_t - 1))
                            nmm += 1
                    segT = segpool.tile([128, 128], bf16, tag="segT")
                    nc.scalar.copy(out=segT[:], in_=ps_S[:])

                    ps_Y = psYpool.tile([128, 2 * D], f32, tag="psY")
                    nc.tensor.matmul(ps_Y[:],
                                     lhsT=hT_sb[:, t * 128 : (t + 1) * 128],
                                     rhs=rhsA_sb[:], start=True, stop=False)
                    nc.tensor.matmul(ps_Y[:],
                                     lhsT=onesp_sb[:, t * 128 : (t + 1) * 128],
                                     rhs=biasAB_sb[:], start=False, stop=False)
                    nc.tensor.matmul(ps_Y[:], lhsT=segT[:], rhs=rhsB_sb[:],
                                     start=False, stop=True)

                    # gate = sigmoid(Y[:, D:]) ; t1 = m - h
                    nc.scalar.activation(out=g_slab[:, ti, :], in_=ps_Y[:, D:],
                                         func=Act.Sigmoid)
                    nc.vector.tensor_tensor(out=t1_slab[:, ti, :],
                                            in0=ps_Y[:, :D], in1=h_g[:, ti, :],
                                            op=Alu.subtract)

                # ---- group-batched elementwise + LayerNorm ----
                nc.vector.tensor_tensor(out=t2_slab[:], in0=g_slab[:],
                                        in1=t1_slab[:], op=Alu.mult)
                nc.vector.tensor_tensor(out=v_slab[:], in0=t2_slab[:],
                                        in1=h_g[:], op=Alu.add)
                sumv = spool.tile([128, TG], f32, tag="sumv")
                nc.vector.tensor_reduce(out=sumv[:], in_=v_slab[:],
                                        axis=Axis.X, op=Alu.add)
                nc.vector.tensor_tensor(out=vsq_slab[:], in0=v_slab[:],
                                        in1=v_slab[:], op=Alu.mult)
                sv2 = spool.tile([128, TG], f32, tag="sv2")
                nc.vector.tensor_reduce(out=sv2[:], in_=vsq_slab[:],
                                        axis=Axis.X, op=Alu.add)
                mu = spool.tile([128, TG], f32, tag="mu")
                nc.vector.tensor_scalar(out=mu[:], in0=sumv[:],
                                        scalar1=1.0 / D, scalar2=None,
                                        op0=Alu.mult)
                meansq = spool.tile([128, TG], f32, tag="meansq")
                nc.vector.tensor_scalar(out=meansq[:], in0=sv2[:],
                                        scalar1=1.0 / D, scalar2=None,
                                        op0=Alu.mult)
                musq = spool.tile([128, TG], f32, tag="musq")
                nc.vector.tensor_tensor(out=musq[:], in0=mu[:], in1=mu[:],
                                        op=Alu.mult)
                var = spool.tile([128, TG], f32, tag="var")
                nc.vector.tensor_tensor(out=var[:], in0=meansq[:], in1=musq[:],
                                        op=Alu.subtract)
                std = spool.tile([128, TG], f32, tag="std")
                nc.scalar.activation(out=std[:], in_=var[:], func=Act.Sqrt,
                                     bias=eps_sb[:], scale=1.0)
                rstd = spool.tile([128, TG], f32, tag="rstd")
                nc.vector.reciprocal(out=rstd[:], in_=std[:])
                bias_ln = spool.tile([128, TG], f32, tag="bias_ln")
                nc.vector.scalar_tensor_tensor(
                    out=bias_ln[:], in0=mu[:], scalar=-1.0, in1=rstd[:],
                    op0=Alu.mult, op1=Alu.mult)

                for ti in range(TG):
                    if trivial_ln:
                        nc.scalar.activation(
                            out=out_stage[:, ti, :], in_=v_slab[:, ti, :],
                            func=Act.Relu, bias=bias_ln[:, ti : ti + 1],
                            scale=rstd[:, ti : ti + 1])
                    else:
                        z = segpool.tile([128, D], f32, tag="z")
                        nc.scalar.activation(
                            out=z[:], in_=v_slab[:, ti, :], func=Act.Identity,
                            bias=bias_ln[:, ti : ti + 1],
                            scale=rstd[:, ti : ti + 1])
                        nc.vector.tensor_tensor(out=z[:], in0=z[:],
                                                in1=gamma_sb[:], op=Alu.mult)
                        nc.vector.tensor_tensor(out=z[:], in0=z[:],
                                                in1=beta_sb[:], op=Alu.add)
                        nc.scalar.activation(out=out_stage[:, ti, :], in_=z[:],
                                             func=Act.Relu)

                nc.scalar.dma_start(
                    out=bass.AP(tensor=out_d, offset=g * TG * 128 * D,
                                ap=[[D, 128], [128 * D, TG], [1, D]]),
                    in_=out_stage[:])
    nc.compile()
    return nc


def kernel(**inputs):
    from concourse.bass_utils import run_bass_kernel_spmd

    per_core, meta = _prep(**{k: np.asarray(v) for k, v in inputs.items()})
    idxcols = per_core[0]["idx_all"].shape[1]
    key = (tuple(meta["caps"]), meta["trivial_ln"], idxcols)
    if key not in _BASS_CACHE:
        _BASS_CACHE[key] = _build_bass(
            meta["caps"], meta["S_t"], meta["cap_off"], meta["trivial_ln"],
            idxcols)
    nc = _BASS_CACHE[key]

    in_maps = []
    for pc in per_core:
        m = dict(pc)
        if not meta["trivial_ln"]:
            m["ln_gamma"] = meta["ln_gamma"][None]
            m["ln_beta"] = meta["ln_beta"][None]
        in_maps.append(m)
    res = run_bass_kernel_spmd(nc, in_maps, core_ids=list(range(N_CORES)))
    outs = [res.results[c]["out"] for c in range(N_CORES)]
    return np.concatenate(outs, 0)[:N].astype(np.float32)


# revision 7
# speedup vs baseline: 3.6689x; 1.4617x over previous
"""GNN message-passing layer (DAGLayer) on 8 Trainium2 NeuronCores.

Strategy (v2):
  - Shard destination nodes 8 ways (12544 nodes = 98 tiles of 128 per core,
    N padded 100000 -> 100352). Replicate h (as a bf16 gather table) so
    cross-partition edges need no communication.
  - Segment-sum over edges via TensorE matmuls: for each 128-dst tile,
    gather h[src] rows (dma_gather, bf16) for its edges in chunks of 128,
    multiply with host-prebuilt one-hot planes PT (scaled by 1/count so the
    matmul yields the neighbor MEAN directly), accumulate segT = G^T @ PT
    in PSUM ([dim, dst] layout).
  - The GpSimd engine queue carries ONLY the dma_gathers so descriptor
    generation (the bottleneck) overlaps all compute.
  - Dense path per tile with host-folded weights, all bf16, accumulated in
    one PSUM bank:
      PSUM_Y = h @ [Ws^T | (Wg1+Wg2@Ws)^T] + [1;p]^T @ [[b_s|b_g+Wg2@b_s],
               [b_n|Wg2@b_n]] + seg_mean @ [Wn^T | (Wg2@Wn)^T]
      m, gate_pre = Y[:, :128], Y[:, 128:]
      v = h + sigmoid(gate_pre) * (m - h)
      out = relu((v - mu) * rstd)   (LayerNorm, batched per group of 7
      tiles on DVE with a single Rsqrt activation)
"""

import math
import numpy as np

try:
    import ml_dtypes

    BF16 = np.dtype(ml_dtypes.bfloat16)
except ImportError:  # pragma: no cover
    BF16 = None

N = 100000
E = 600000
D = 128
N_CORES = 8
NPC = 12544            # nodes per core
TPC = NPC // 128       # 98 tiles per core
N_PAD = NPC * N_CORES  # 100352
N_BANKS = 4
BANK = N_PAD // N_BANKS  # 25088
TG = 7                 # tiles per gather group
N_GROUPS = TPC // TG   # 14
LN_EPS = 1e-5


def _build_schedule(edge_src, edge_dst):
    """Chunk/slot schedule. Returns per-core idx16 arrays, per-core PT
    one-hot planes (scaled by 1/count), and the slot capacities."""
    edge_src = np.asarray(edge_src, np.int64)
    edge_dst = np.asarray(edge_dst, np.int64)

    tile_g = edge_dst // 128          # global dst tile 0..783
    bank = edge_src // BANK           # 0..3
    key = tile_g * N_BANKS + bank
    order = np.argsort(key, kind="stable")
    src_s = edge_src[order]
    dst_s = edge_dst[order]

    n_tiles_g = N_PAD // 128          # 784
    cnt = np.bincount(key[order], minlength=n_tiles_g * N_BANKS).reshape(
        n_tiles_g, N_BANKS
    )
    starts = np.zeros((n_tiles_g, N_BANKS), np.int64)
    starts.reshape(-1)[1:] = np.cumsum(cnt.reshape(-1))[:-1]

    caps = [max(1, int(np.ceil(cnt[:, b].max() / 128.0))) for b in range(N_BANKS)]
    S_t = sum(caps)                   # slots per tile
    cap_off = np.cumsum([0] + caps)   # slot offset of bank b within a tile

    counts_n = np.bincount(edge_dst, minlength=N_PAD).astype(np.float64)
    inv_cnt = np.zeros(N_PAD, np.float32)
    nz = counts_n > 0
    inv_cnt[nz] = (1.0 / counts_n[nz]).astype(np.float32)

    idx16 = np.zeros((N_CORES, N_GROUPS, N_BANKS), object)
    pt_all = np.zeros((N_CORES, 128, TPC * S_t * 128), np.float32)

    for c in range(N_CORES):
        pt_c = pt_all[c]
        for g in range(N_GROUPS):
            for b in range(N_BANKS):
                lst = np.zeros(TG * caps[b] * 128, np.int16)
                for ti in range(TG):
                    t_local = g * TG + ti
                    t_glob = c * TPC + t_local
                    n_e = int(cnt[t_glob, b])
                    s0 = int(starts[t_glob, b])
                    srcs = src_s[s0 : s0 + n_e] - b * BANK
                    dsts = dst_s[s0 : s0 + n_e]
                    base = ti * caps[b] * 128
                    lst[base : base + n_e] = srcs.astype(np.int16)
                    pos = np.arange(n_e)
                    rows = pos % 128
                    js = pos // 128
                    slot = t_local * S_t + cap_off[b] + js
                    cols = slot * 128 + (dsts - t_glob * 128)
                    pt_c[rows, cols] = inv_cnt[dsts]
                idx16[c, g, b] = lst
    return idx16, pt_all, caps, S_t, cap_off


def _wrap_idx16(flat):
    """dma_gather index layout: idx i -> [i % 16, i // 16], replicated to
    all 128 partitions."""
    n = len(flat)
    w = np.zeros((128, n // 16), np.int16)
    w16 = flat.reshape(n // 16, 16).T  # [16, n/16]
    w[:16] = w16
    w[16:] = np.tile(w16, (7, 1))
    return w


def _fold_weights(W_self, b_self, W_neigh, b_neigh, W_gate, b_gate):
    Wg1 = W_gate[:, :D]
    Wg2 = W_gate[:, D:]
    rhs_A = np.concatenate([W_self.T, (Wg1 + Wg2 @ W_self).T], 1)  # [128,256]
    rhs_B = np.concatenate([W_neigh.T, (Wg2 @ W_neigh).T], 1)      # [128,256]
    bias_A = np.concatenate([b_self, b_gate + Wg2 @ b_self])[None]  # [1,256]
    bias_Bp = np.concatenate([b_neigh, Wg2 @ b_neigh])[None]        # [1,256]
    return (
        np.ascontiguousarray(rhs_A, np.float32),
        np.ascontiguousarray(rhs_B, np.float32),
        np.ascontiguousarray(bias_A, np.float32),
        np.ascontiguousarray(bias_Bp, np.float32),
    )


def _prep(h, edge_src, edge_dst, W_self, b_self, W_neigh, b_neigh, W_gate,
          b_gate, ln_gamma, ln_beta):
    h = np.asarray(h, np.float32)
    h_pad = np.zeros((N_PAD, D), np.float32)
    h_pad[:N] = h
    h_bf16 = h_pad.astype(BF16)

    idx16, pt_all, caps, S_t, cap_off = _build_schedule(edge_src, edge_dst)

    counts = np.bincount(np.asarray(edge_dst, np.int64), minlength=N_PAD)
    p = (counts > 0).astype(np.float32)

    rhs_A, rhs_B, bias_A, bias_Bp = _fold_weights(
        np.asarray(W_self, np.float32), np.asarray(b_self, np.float32),
        np.asarray(W_neigh, np.float32), np.asarray(b_neigh, np.float32),
        np.asarray(W_gate, np.float32), np.asarray(b_gate, np.float32))
    bias_AB = np.concatenate([bias_A, bias_Bp], 0)  # [2, 256]

    trivial_ln = (np.allclose(ln_gamma, 1.0) and np.allclose(ln_beta, 0.0))

    per_core = []
    for c in range(N_CORES):
        rows = slice(c * NPC, (c + 1) * NPC)
        idx_segs = []
        for g in range(N_GROUPS):
            for b in range(N_BANKS):
                idx_segs.append(_wrap_idx16(idx16[c, g, b]))
        idx_all = np.concatenate(idx_segs, axis=1)  # [128, sum(n/16)]
        ones_p = np.stack([np.ones(NPC, np.float32), p[rows]], 0)  # [2, NPC]
        per_core.append(dict(
            h_self=np.ascontiguousarray(h_bf16[rows]),
            hT_self=np.ascontiguousarray(h_bf16[rows].T),
            h_gather=h_bf16,
            idx_all=idx_all,
            pt_all=np.ascontiguousarray(pt_all[c].astype(BF16)),
            ones_p=np.ascontiguousarray(ones_p.astype(BF16)),
            rhs_A=np.ascontiguousarray(rhs_A.astype(BF16)),
            rhs_B=np.ascontiguousarray(rhs_B.astype(BF16)),
            bias_AB=np.ascontiguousarray(bias_AB.astype(BF16)),
        ))
    meta = dict(caps=caps, S_t=S_t, cap_off=cap_off, trivial_ln=trivial_ln,
                ln_gamma=np.asarray(ln_gamma, np.float32),
                ln_beta=np.asarray(ln_beta, np.float32))
    return per_core, meta


# ---------------------------------------------------------------------------
# Bass device kernel
# ---------------------------------------------------------------------------

_BASS_CACHE = {}


def _build_bass(caps, S_t, cap_off, trivial_ln, idxcols):
    import concourse.bacc as bacc
    import concourse.bass as bass
    import concourse.tile as tile
    from concourse import mybir

    f32 = mybir.dt.float32
    bf16 = mybir.dt.bfloat16
    i16 = mybir.dt.int16
    Alu = mybir.AluOpType
    Act = mybir.ActivationFunctionType
    Axis = mybir.AxisListType

    nc = bacc.Bacc("TRN2", target_bir_lowering=False, debug=False,
                   num_devices=N_CORES, num_swdge_queues=4)

    h_self = nc.dram_tensor("h_self", [NPC, D], bf16, kind="ExternalInput")
    hT_self = nc.dram_tensor("hT_self", [D, NPC], bf16, kind="ExternalInput")
    h_gather = nc.dram_tensor("h_gather", [N_PAD, D], bf16, kind="ExternalInput")
    idx_all = nc.dram_tensor("idx_all", [128, idxcols], i16, kind="ExternalInput")
    pt_all_d = nc.dram_tensor("pt_all", [128, TPC * S_t * 128], bf16,
                              kind="ExternalInput")
    ones_p_d = nc.dram_tensor("ones_p", [2, NPC], bf16, kind="ExternalInput")
    rhs_A_d = nc.dram_tensor("rhs_A", [D, 2 * D], bf16, kind="ExternalInput")
    rhs_B_d = nc.dram_tensor("rhs_B", [D, 2 * D], bf16, kind="ExternalInput")
    bias_AB_d = nc.dram_tensor("bias_AB", [2, 2 * D], bf16, kind="ExternalInput")
    out_d = nc.dram_tensor("out", [NPC, D], f32, kind="ExternalOutput")

    b_off = [TG * sum(caps[:b]) for b in range(N_BANKS)]  # G slot region start
    seg_w = [TG * caps[b] * 128 // 16 for b in range(N_BANKS)]  # idx cols/(g,b)

    with tile.TileContext(nc) as tc:
        with (
            tc.tile_pool(name="consts", bufs=1) as cpool,
            tc.tile_pool(name="gbuf", bufs=2) as gpool,
            tc.tile_pool(name="ptbuf", bufs=2) as ptpool,
            tc.tile_pool(name="hbuf", bufs=2) as hpool,
            tc.tile_pool(name="obuf", bufs=2) as opool,
            tc.tile_pool(name="slab", bufs=2) as lpool,
            tc.tile_pool(name="small", bufs=2) as spool,
            tc.tile_pool(name="seg", bufs=3) as segpool,
            tc.tile_pool(name="psS", bufs=2, space="PSUM") as psSpool,
            tc.tile_pool(name="psY", bufs=3, space="PSUM") as psYpool,
        ):
            # --- constants, loaded once ---
            idx_sb = cpool.tile([128, idxcols], i16)
            nc.sync.dma_start(out=idx_sb[:], in_=idx_all[:])
            hT_sb = cpool.tile([D, NPC], bf16)
            nc.sync.dma_start(out=hT_sb[:], in_=hT_self[:])
            onesp_sb = cpool.tile([2, NPC], bf16)
            nc.sync.dma_start(out=onesp_sb[:], in_=ones_p_d[:])
            rhsA_sb = cpool.tile([D, 2 * D], bf16)
            nc.sync.dma_start(out=rhsA_sb[:], in_=rhs_A_d[:])
            rhsB_sb = cpool.tile([D, 2 * D], bf16)
            nc.sync.dma_start(out=rhsB_sb[:], in_=rhs_B_d[:])
            biasAB_sb = cpool.tile([2, 2 * D], bf16)
            nc.sync.dma_start(out=biasAB_sb[:], in_=bias_AB_d[:])
            eps_sb = cpool.tile([128, 1], f32)
            nc.vector.memset(eps_sb[:], LN_EPS)
            if not trivial_ln:
                gamma_sb = cpool.tile([128, D], f32)
                nc.gpsimd.dma_start(
                    out=gamma_sb[:],
                    in_=bass.AP(tensor=nc.dram_tensor(
                        "ln_gamma", [1, D], f32, kind="ExternalInput"),
                        offset=0, ap=[[0, 128], [1, D]]))
                beta_sb = cpool.tile([128, D], f32)
                nc.gpsimd.dma_start(
                    out=beta_sb[:],
                    in_=bass.AP(tensor=nc.dram_tensor(
                        "ln_beta", [1, D], f32, kind="ExternalInput"),
                        offset=0, ap=[[0, 128], [1, D]]))

            idx_off = [0]
            for g in range(N_GROUPS):
                for b in range(N_BANKS):
                    idx_off.append(idx_off[-1] + seg_w[b])

            for g in range(N_GROUPS):
                # ---- group loads ----
                G = gpool.tile([128, TG * S_t, 128], bf16, tag="G")
                for b in range(N_BANKS):
                    o = idx_off[g * N_BANKS + b]
                    nidx = TG * caps[b] * 128
                    nc.gpsimd.dma_gather(
                        out_ap=G[:, b_off[b] : b_off[b] + TG * caps[b], :],
                        in_ap=h_gather[b * BANK :, :],
                        idxs_ap=idx_sb[:, o : o + seg_w[b]],
                        num_idxs=nidx,
                        num_idxs_reg=nidx,
                        elem_size=D,
                        single_packet=False,
                        queue_num=b,
                    )
                PTt = ptpool.tile([128, TG * S_t * 128], bf16, tag="PT")
                nc.sync.dma_start(
                    out=PTt[:],
                    in_=pt_all_d[:, g * TG * S_t * 128 : (g + 1) * TG * S_t * 128])
                h_g = hpool.tile([128, TG, 128], bf16, tag="hg")
                nc.sync.dma_start(
                    out=h_g[:],
                    in_=bass.AP(tensor=h_self, offset=g * TG * 128 * D,
                                ap=[[D, 128], [128 * D, TG], [1, D]]))

                out_stage = opool.tile([128, TG, 128], f32, tag="ostage")
                t1_slab = lpool.tile([128, TG, 128], f32, tag="t1")
                g_slab = lpool.tile([128, TG, 128], bf16, tag="gs")
                t2_slab = lpool.tile([128, TG, 128], f32, tag="t2")
                v_slab = lpool.tile([128, TG, 128], f32, tag="v")
                vsq_slab = lpool.tile([128, TG, 128], f32, tag="vsq")

                # ---- per-tile matmuls + PSUM-adjacent ops ----
                for ti in range(TG):
                    t = g * TG + ti
                    ps_S = psSpool.tile([128, 128], f32, tag="psS")
                    nmm = 0
                    for b in range(N_BANKS):
                        for j in range(caps[b]):
                            slot = b_off[b] + ti * caps[b] + j
                            k = cap_off[b] + j
                            col = (ti * S_t + k) * 128
                            nc.tensor.matmul(
                                ps_S[:], lhsT=G[:, slot, :],
                                rhs=PTt[:, col : col + 128],
                                start=(nmm == 0), stop=(nmm == S_t - 1))
                            nmm += 1
                    segT = segpool.tile([128, 128], bf16, tag="segT")
                    nc.scalar.copy(out=segT[:], in_=ps_S[:])

                    ps_Y = psYpool.tile([128, 2 * D], f32, tag="psY")
                    nc.tensor.matmul(ps_Y[:],
                                     lhsT=hT_sb[:, t * 128 : (t + 1) * 128],
                                     rhs=rhsA_sb[:], start=True, stop=False)
                    nc.tensor.matmul(ps_Y[:],
                                     lhsT=onesp_sb[:, t * 128 : (t + 1) * 128],
                                     rhs=biasAB_sb[:], start=False, stop=False)
                    nc.tensor.matmul(ps_Y[:], lhsT=segT[:], rhs=rhsB_sb[:],
                                     start=False, stop=True)

                    # gate = sigmoid(Y[:, D:]) ; t1 = m - h
                    nc.scalar.activation(out=g_slab[:, ti, :], in_=ps_Y[:, D:],
                                         func=Act.Sigmoid)
                    nc.vector.tensor_tensor(out=t1_slab[:, ti, :],
                                            in0=ps_Y[:, :D], in1=h_g[:, ti, :],
                                            op=Alu.subtract)

                # ---- group-batched elementwise + LayerNorm ----
                nc.vector.tensor_tensor(out=t2_slab[:], in0=g_slab[:],
                                        in1=t1_slab[:], op=Alu.mult)
                nc.vector.tensor_tensor(out=v_slab[:], in0=t2_slab[:],
                                        in1=h_g[:], op=Alu.add)
                sumv = spool.tile([128, TG], f32, tag="sumv")
                nc.vector.tensor_reduce(out=sumv[:], in_=v_slab[:],
                                        axis=Axis.X, op=Alu.add)
                nc.vector.tensor_tensor(out=vsq_slab[:], in0=v_slab[:],
                                        in1=v_slab[:], op=Alu.mult)
                sv2 = spool.tile([128, TG], f32, tag="sv2")
                nc.vector.tensor_reduce(out=sv2[:], in_=vsq_slab[:],
                                        axis=Axis.X, op=Alu.add)
                mu = spool.tile([128, TG], f32, tag="mu")
                nc.vector.tensor_scalar(out=mu[:], in0=sumv[:],
                                        scalar1=1.0 / D, scalar2=None,
                                        op0=Alu.mult)
                meansq = spool.tile([128, TG], f32, tag="meansq")
                nc.vector.tensor_scalar(out=meansq[:], in0=sv2[:],
                                        scalar1=1.0 / D, scalar2=None,
                                        op0=Alu.mult)
                musq = spool.tile([128, TG], f32, tag="musq")
                nc.vector.tensor_tensor(out=musq[:], in0=mu[:], in1=mu[:],
                                        op=Alu.mult)
                var = spool.tile([128, TG], f32, tag="var")
                nc.vector.tensor_tensor(out=var[:], in0=meansq[:], in1=musq[:],
                                        op=Alu.subtract)
                std = spool.tile([128, TG], f32, tag="std")
                nc.scalar.activation(out=std[:], in_=var[:], func=Act.Sqrt,
                                     bias=eps_sb[:], scale=1.0)
                rstd = spool.tile([128, TG], f32, tag="rstd")
                nc.vector.reciprocal(out=rstd[:], in_=std[:])
                bias_ln = spool.tile([128, TG], f32, tag="bias_ln")
                nc.vector.scalar_tensor_tensor(
                    out=bias_ln[:], in0=mu[:], scalar=-1.0, in1=rstd[:],
                    op0=Alu.mult, op1=Alu.mult)

                for ti in range(TG):
                    if trivial_ln:
                        nc.scalar.activation(
                            out=out_stage[:, ti, :], in_=v_slab[:, ti, :],
                            func=Act.Relu, bias=bias_ln[:, ti : ti + 1],
                            scale=rstd[:, ti : ti + 1])
                    else:
                        z = segpool.tile([128, D], f32, tag="z")
                        nc.scalar.activation(
                            out=z[:], in_=v_slab[:, ti, :], func=Act.Identity,
                            bias=bias_ln[:, ti : ti + 1],
                            scale=rstd[:, ti : ti + 1])
                        nc.vector.tensor_tensor(out=z[:], in0=z[:],
                                                in1=gamma_sb[:], op=Alu.mult)
                        nc.vector.tensor_tensor(out=z[:], in0=z[:],
                                                in1=beta_sb[:], op=Alu.add)
                        nc.scalar.activation(out=out_stage[:, ti, :], in_=z[:],
                                             func=Act.Relu)

                nc.scalar.dma_start(
                    out=bass.AP(tensor=out_d, offset=g * TG * 128 * D,
                                ap=[[D, 128], [128 * D, TG], [1, D]]),
                    in_=out_stage[:])
    nc.compile()
    return nc


def kernel(**inputs):
    from concourse.bass_utils import run_bass_kernel_spmd

    per_core, meta = _prep(**{k: np.asarray(v) for k, v in inputs.items()})
    idxcols = per_core[0]["idx_all"].shape[1]
    key = (tuple(meta["caps"]), meta["trivial_ln"], idxcols)
    if key not in _BASS_CACHE:
        _BASS_CACHE[key] = _build_bass(
            meta["caps"], meta["S_t"], meta["cap_off"], meta["trivial_ln"],
            idxcols)
    nc = _BASS_CACHE[key]

    in_maps = []
    for pc in per_core:
        m = dict(pc)
        if not meta["trivial_ln"]:
            m["ln_gamma"] = meta["ln_gamma"][None]
            m["ln_beta"] = meta["ln_beta"][None]
        in_maps.append(m)
    res = run_bass_kernel_spmd(nc, in_maps, core_ids=list(range(N_CORES)))
    outs = [res.results[c]["out"] for c in range(N_CORES)]
    return np.concatenate(outs, 0)[:N].astype(np.float32)


# revision 13
# speedup vs baseline: 3.8308x; 1.0441x over previous
"""GNN message-passing layer (DAGLayer) on 8 Trainium2 NeuronCores.

Strategy (v2):
  - Shard destination nodes 8 ways (12544 nodes = 98 tiles of 128 per core,
    N padded 100000 -> 100352). Replicate h (as a bf16 gather table) so
    cross-partition edges need no communication.
  - Segment-sum over edges via TensorE matmuls: for each 128-dst tile,
    gather h[src] rows (dma_gather, bf16) for its edges in chunks of 128,
    multiply with host-prebuilt one-hot planes PT (scaled by 1/count so the
    matmul yields the neighbor MEAN directly), accumulate segT = G^T @ PT
    in PSUM ([dim, dst] layout).
  - The GpSimd engine queue carries ONLY the dma_gathers so descriptor
    generation (the bottleneck) overlaps all compute.
  - Dense path per tile with host-folded weights, all bf16, accumulated in
    one PSUM bank:
      PSUM_Y = h @ [Ws^T | (Wg1+Wg2@Ws)^T] + [1;p]^T @ [[b_s|b_g+Wg2@b_s],
               [b_n|Wg2@b_n]] + seg_mean @ [Wn^T | (Wg2@Wn)^T]
      m, gate_pre = Y[:, :128], Y[:, 128:]
      v = h + sigmoid(gate_pre) * (m - h)
      out = relu((v - mu) * rstd)   (LayerNorm, batched per group of 7
      tiles on DVE with a single Rsqrt activation)
"""

import math
import numpy as np

try:
    import ml_dtypes

    BF16 = np.dtype(ml_dtypes.bfloat16)
    FP8 = np.dtype(ml_dtypes.float8_e4m3)
except ImportError:  # pragma: no cover
    BF16 = None
    FP8 = None

N = 100000
E = 600000
D = 128
N_CORES = 8
NPC = 12544            # nodes per core
TPC = NPC // 128       # 98 tiles per core
N_PAD = NPC * N_CORES  # 100352
N_BANKS = 4
BANK = N_PAD // N_BANKS  # 25088
TG = 7                 # tiles per gather group
N_GROUPS = TPC // TG   # 14
LN_EPS = 1e-5


def _build_schedule(edge_src, edge_dst):
    """Chunk/slot schedule. Returns per-core idx16 arrays, per-core PT
    one-hot planes (scaled by 1/count), and the slot capacities."""
    edge_src = np.asarray(edge_src, np.int64)
    edge_dst = np.asarray(edge_dst, np.int64)

    tile_g = edge_dst // 128          # global dst tile 0..783
    bank = edge_src // BANK           # 0..3
    key = tile_g * N_BANKS + bank
    order = np.argsort(key, kind="stable")
    src_s = edge_src[order]
    dst_s = edge_dst[order]

    n_tiles_g = N_PAD // 128          # 784
    cnt = np.bincount(key[order], minlength=n_tiles_g * N_BANKS).reshape(
        n_tiles_g, N_BANKS
    )
    starts = np.zeros((n_tiles_g, N_BANKS), np.int64)
    starts.reshape(-1)[1:] = np.cumsum(cnt.reshape(-1))[:-1]

    caps = [max(1, int(np.ceil(cnt[:, b].max() / 128.0))) for b in range(N_BANKS)]
    S_t = sum(caps)                   # slots per tile
    cap_off = np.cumsum([0] + caps)   # slot offset of bank b within a tile

    counts_n = np.bincount(edge_dst, minlength=N_PAD).astype(np.float64)
    inv_cnt = np.zeros(N_PAD, np.float32)
    nz = counts_n > 0
    inv_cnt[nz] = (1.0 / counts_n[nz]).astype(np.float32)

    idx16 = np.zeros((N_CORES, N_GROUPS, N_BANKS), object)
    pt_all = np.zeros((N_CORES, 128, TPC * S_t * 128), np.float32)

    for c in range(N_CORES):
        pt_c = pt_all[c]
        for g in range(N_GROUPS):
            for b in range(N_BANKS):
                lst = np.zeros(TG * caps[b] * 128, np.int16)
                for ti in range(TG):
                    t_local = g * TG + ti
                    t_glob = c * TPC + t_local
                    n_e = int(cnt[t_glob, b])
                    s0 = int(starts[t_glob, b])
                    srcs = src_s[s0 : s0 + n_e] - b * BANK
                    dsts = dst_s[s0 : s0 + n_e]
                    base = ti * caps[b] * 128
                    lst[base : base + n_e] = srcs.astype(np.int16)
                    pos = np.arange(n_e)
                    rows = pos % 128
                    js = pos // 128
                    slot = t_local * S_t + cap_off[b] + js
                    cols = slot * 128 + (dsts - t_glob * 128)
                    pt_c[rows, cols] = inv_cnt[dsts]
                idx16[c, g, b] = lst
    return idx16, pt_all, caps, S_t, cap_off


def _wrap_idx16(flat):
    """dma_gather index layout: idx i -> [i % 16, i // 16], replicated to
    all 128 partitions."""
    n = len(flat)
    w = np.zeros((128, n // 16), np.int16)
    w16 = flat.reshape(n // 16, 16).T  # [16, n/16]
    w[:16] = w16
    w[16:] = np.tile(w16, (7, 1))
    return w


def _fold_weights(W_self, b_self, W_neigh, b_neigh, W_gate, b_gate):
    Wg1 = W_gate[:, :D]
    Wg2 = W_gate[:, D:]
    rhs_A = np.concatenate([W_self.T, (Wg1 + Wg2 @ W_self).T], 1)  # [128,256]
    rhs_B = np.concatenate([W_neigh.T, (Wg2 @ W_neigh).T], 1)      # [128,256]
    bias_A = np.concatenate([b_self, b_gate + Wg2 @ b_self])[None]  # [1,256]
    bias_Bp = np.concatenate([b_neigh, Wg2 @ b_neigh])[None]        # [1,256]
    return (
        np.ascontiguousarray(rhs_A, np.float32),
        np.ascontiguousarray(rhs_B, np.float32),
        np.ascontiguousarray(bias_A, np.float32),
        np.ascontiguousarray(bias_Bp, np.float32),
    )


def _prep(h, edge_src, edge_dst, W_self, b_self, W_neigh, b_neigh, W_gate,
          b_gate, ln_gamma, ln_beta):
    h = np.asarray(h, np.float32)
    h_pad = np.zeros((N_PAD, D), np.float32)
    h_pad[:N] = h
    h_bf16 = h_pad.astype(BF16)

    idx16, pt_all, caps, S_t, cap_off = _build_schedule(edge_src, edge_dst)

    counts = np.bincount(np.asarray(edge_dst, np.int64), minlength=N_PAD)
    p = (counts > 0).astype(np.float32)

    rhs_A, rhs_B, bias_A, bias_Bp = _fold_weights(
        np.asarray(W_self, np.float32), np.asarray(b_self, np.float32),
        np.asarray(W_neigh, np.float32), np.asarray(b_neigh, np.float32),
        np.asarray(W_gate, np.float32), np.asarray(b_gate, np.float32))
    bias_AB = np.concatenate([bias_A, bias_Bp], 0)  # [2, 256]

    trivial_ln = (np.allclose(ln_gamma, 1.0) and np.allclose(ln_beta, 0.0))

    per_core = []
    for c in range(N_CORES):
        rows = slice(c * NPC, (c + 1) * NPC)
        idx_segs = []
        for g in range(N_GROUPS):
            for b in range(N_BANKS):
                idx_segs.append(_wrap_idx16(idx16[c, g, b]))
        idx_all = np.concatenate(idx_segs, axis=1)  # [128, sum(n/16)]
        ones_p = np.stack([np.ones(NPC, np.float32), p[rows]], 0)  # [2, NPC]
        per_core.append(dict(
            h_self=np.ascontiguousarray(h_bf16[rows]),
            hT_self=np.ascontiguousarray(h_bf16[rows].T),
            h_gather=h_bf16,
            idx_all=idx_all,
            pt_all=np.ascontiguousarray(pt_all[c].astype(FP8)),
            ones_p=np.ascontiguousarray(ones_p.astype(BF16)),
            rhs_A=np.ascontiguousarray(rhs_A.astype(BF16)),
            rhs_B=np.ascontiguousarray(rhs_B.astype(BF16)),
            bias_AB=np.ascontiguousarray(bias_AB.astype(BF16)),
        ))
    meta = dict(caps=caps, S_t=S_t, cap_off=cap_off, trivial_ln=trivial_ln,
                ln_gamma=np.asarray(ln_gamma, np.float32),
                ln_beta=np.asarray(ln_beta, np.float32))
    return per_core, meta


# ---------------------------------------------------------------------------
# Bass device kernel
# ---------------------------------------------------------------------------

_BASS_CACHE = {}


def _build_bass(caps, S_t, cap_off, trivial_ln, idxcols):
    import concourse.bacc as bacc
    import concourse.bass as bass
    import concourse.tile as tile
    from concourse import mybir

    f32 = mybir.dt.float32
    bf16 = mybir.dt.bfloat16
    i16 = mybir.dt.int16
    Alu = mybir.AluOpType
    Act = mybir.ActivationFunctionType
    Axis = mybir.AxisListType

    nc = bacc.Bacc("TRN2", target_bir_lowering=False, debug=False,
                   num_devices=N_CORES, num_swdge_queues=4)

    fp8 = mybir.dt.float8e4

    h_self = nc.dram_tensor("h_self", [NPC, D], bf16, kind="ExternalInput")
    hT_self = nc.dram_tensor("hT_self", [D, NPC], bf16, kind="ExternalInput")
    h_gather = nc.dram_tensor("h_gather", [N_PAD, D], bf16, kind="ExternalInput")
    idx_all = nc.dram_tensor("idx_all", [128, idxcols], i16, kind="ExternalInput")
    pt_all_d = nc.dram_tensor("pt_all", [128, TPC * S_t * 128], fp8,
                              kind="ExternalInput")
    ones_p_d = nc.dram_tensor("ones_p", [2, NPC], bf16, kind="ExternalInput")
    rhs_A_d = nc.dram_tensor("rhs_A", [D, 2 * D], bf16, kind="ExternalInput")
    rhs_B_d = nc.dram_tensor("rhs_B", [D, 2 * D], bf16, kind="ExternalInput")
    bias_AB_d = nc.dram_tensor("bias_AB", [2, 2 * D], bf16, kind="ExternalInput")
    out_d = nc.dram_tensor("out", [NPC, D], bf16, kind="ExternalOutput")

    b_off = [TG * sum(caps[:b]) for b in range(N_BANKS)]  # G slot region start
    seg_w = [TG * caps[b] * 128 // 16 for b in range(N_BANKS)]  # idx cols/(g,b)

    with tile.TileContext(nc) as tc:
        with (
            tc.tile_pool(name="consts", bufs=1) as cpool,
            tc.tile_pool(name="gbuf", bufs=2) as gpool,
            tc.tile_pool(name="ptbuf", bufs=2) as ptpool,
            tc.tile_pool(name="hbuf", bufs=2) as hpool,
            tc.tile_pool(name="obuf", bufs=2) as opool,
            tc.tile_pool(name="slab", bufs=2) as lpool,
            tc.tile_pool(name="small", bufs=2) as spool,
            tc.tile_pool(name="seg", bufs=3) as segpool,
            tc.tile_pool(name="psS", bufs=2, space="PSUM") as psSpool,
            tc.tile_pool(name="psY", bufs=3, space="PSUM") as psYpool,
        ):
            # --- constants, loaded once ---
            idx_sb = cpool.tile([128, idxcols], i16)
            nc.sync.dma_start(out=idx_sb[:], in_=idx_all[:])
            hT_sb = cpool.tile([D, NPC], bf16)
            nc.sync.dma_start(out=hT_sb[:], in_=hT_self[:])
            onesp_sb = cpool.tile([2, NPC], bf16)
            nc.sync.dma_start(out=onesp_sb[:], in_=ones_p_d[:])
            rhsA_sb = cpool.tile([D, 2 * D], bf16)
            nc.sync.dma_start(out=rhsA_sb[:], in_=rhs_A_d[:])
            rhsB_sb = cpool.tile([D, 2 * D], bf16)
            nc.sync.dma_start(out=rhsB_sb[:], in_=rhs_B_d[:])
            biasAB_sb = cpool.tile([2, 2 * D], bf16)
            nc.sync.dma_start(out=biasAB_sb[:], in_=bias_AB_d[:])
            eps_sb = cpool.tile([128, 1], f32)
            nc.vector.memset(eps_sb[:], LN_EPS)
            if not trivial_ln:
                gamma_sb = cpool.tile([128, D], f32)
                nc.gpsimd.dma_start(
                    out=gamma_sb[:],
                    in_=bass.AP(tensor=nc.dram_tensor(
                        "ln_gamma", [1, D], f32, kind="ExternalInput"),
                        offset=0, ap=[[0, 128], [1, D]]))
                beta_sb = cpool.tile([128, D], f32)
                nc.gpsimd.dma_start(
                    out=beta_sb[:],
                    in_=bass.AP(tensor=nc.dram_tensor(
                        "ln_beta", [1, D], f32, kind="ExternalInput"),
                        offset=0, ap=[[0, 128], [1, D]]))

            idx_off = [0]
            for g in range(N_GROUPS):
                for b in range(N_BANKS):
                    idx_off.append(idx_off[-1] + seg_w[b])

            def group_tail(gctx):
                """Batched elementwise + LayerNorm + out write for a group."""
                g = gctx["g"]
                h_g = gctx["h_g"]
                t1_slab = gctx["t1"]
                g_slab = gctx["gs"]
                t2_slab = lpool.tile([128, TG, 128], bf16, tag="t2")
                v_slab = lpool.tile([128, TG, 128], bf16, tag="v")
                vsq_slab = lpool.tile([128, TG, 128], f32, tag="vsq")
                out_stage = opool.tile([128, TG, 128], bf16, tag="ostage")
                nc.vector.tensor_tensor(out=t2_slab[:], in0=g_slab[:],
                                        in1=t1_slab[:], op=Alu.mult)
                nc.vector.tensor_tensor(out=v_slab[:], in0=t2_slab[:],
                                        in1=h_g[:], op=Alu.add)
                sumv = spool.tile([128, TG], f32, tag="sumv")
                nc.vector.tensor_reduce(out=sumv[:], in_=v_slab[:],
                                        axis=Axis.X, op=Alu.add)
                nc.vector.tensor_tensor(out=vsq_slab[:], in0=v_slab[:],
                                        in1=v_slab[:], op=Alu.mult)
                sv2 = spool.tile([128, TG], f32, tag="sv2")
                nc.vector.tensor_reduce(out=sv2[:], in_=vsq_slab[:],
                                        axis=Axis.X, op=Alu.add)
                mu = spool.tile([128, TG], f32, tag="mu")
                nc.vector.tensor_scalar(out=mu[:], in0=sumv[:],
                                        scalar1=1.0 / D, scalar2=None,
                                        op0=Alu.mult)
                meansq = spool.tile([128, TG], f32, tag="meansq")
                nc.vector.tensor_scalar(out=meansq[:], in0=sv2[:],
                                        scalar1=1.0 / D, scalar2=None,
                                        op0=Alu.mult)
                musq = spool.tile([128, TG], f32, tag="musq")
                nc.vector.tensor_tensor(out=musq[:], in0=mu[:], in1=mu[:],
                                        op=Alu.mult)
                var = spool.tile([128, TG], f32, tag="var")
                nc.vector.tensor_tensor(out=var[:], in0=meansq[:], in1=musq[:],
                                        op=Alu.subtract)
                std = spool.tile([128, TG], f32, tag="std")
                nc.scalar.activation(out=std[:], in_=var[:], func=Act.Sqrt,
                                     bias=eps_sb[:], scale=1.0)
                rstd = spool.tile([128, TG], f32, tag="rstd")
                nc.vector.reciprocal(out=rstd[:], in_=std[:])
                bias_ln = spool.tile([128, TG], f32, tag="bias_ln")
                nc.vector.scalar_tensor_tensor(
                    out=bias_ln[:], in0=mu[:], scalar=-1.0, in1=rstd[:],
                    op0=Alu.mult, op1=Alu.mult)

                for ti in range(TG):
                    if trivial_ln:
                        nc.scalar.activation(
                            out=out_stage[:, ti, :], in_=v_slab[:, ti, :],
                            func=Act.Relu, bias=bias_ln[:, ti : ti + 1],
                            scale=rstd[:, ti : ti + 1])
                    else:
                        z = segpool.tile([128, D], f32, tag="z")
                        nc.scalar.activation(
                            out=z[:], in_=v_slab[:, ti, :], func=Act.Identity,
                            bias=bias_ln[:, ti : ti + 1],
                            scale=rstd[:, ti : ti + 1])
                        nc.vector.tensor_tensor(out=z[:], in0=z[:],
                                                in1=gamma_sb[:], op=Alu.mult)
                        nc.vector.tensor_tensor(out=z[:], in0=z[:],
                                                in1=beta_sb[:], op=Alu.add)
                        nc.scalar.activation(out=out_stage[:, ti, :], in_=z[:],
                                             func=Act.Relu)

                nc.scalar.dma_start(
                    out=bass.AP(tensor=out_d, offset=g * TG * 128 * D,
                                ap=[[D, 128], [128 * D, TG], [1, D]]),
                    in_=out_stage[:])

            pending = None  # (ps_Y, segT, gctx, ti) awaiting MM_B + consumers
            gctx = None
            for t in range(TPC + 1):
                g, ti = divmod(t, TG)
                if t < TPC:
                    if ti == 0:
                        # ---- group loads ----
                        G = gpool.tile([128, TG * S_t, 128], bf16, tag="G")
                        for b in range(N_BANKS):
                            o = idx_off[g * N_BANKS + b]
                            nidx = TG * caps[b] * 128
                            nc.gpsimd.dma_gather(
                                out_ap=G[:, b_off[b] : b_off[b] + TG * caps[b], :],
                                in_ap=h_gather[b * BANK :, :],
                                idxs_ap=idx_sb[:, o : o + seg_w[b]],
                                num_idxs=nidx,
                                num_idxs_reg=nidx,
                                elem_size=D,
                                single_packet=False,
                                queue_num=b,
                            )
                        PTt = ptpool.tile([128, TG * S_t * 128], bf16, tag="PT")
                        nc.gpsimd.dma_start(
                            out=PTt[:],
                            in_=pt_all_d[:, g * TG * S_t * 128 :
                                         (g + 1) * TG * S_t * 128])
                        h_g = hpool.tile([128, TG, 128], bf16, tag="hg")
                        nc.sync.dma_start(
                            out=h_g[:],
                            in_=bass.AP(tensor=h_self, offset=g * TG * 128 * D,
                                        ap=[[D, 128], [128 * D, TG], [1, D]]))
                        t1_slab = lpool.tile([128, TG, 128], bf16, tag="t1")
                        gs_slab = lpool.tile([128, TG, 128], bf16, tag="gs")
                        gctx = dict(g=g, h_g=h_g, G=G, PTt=PTt,
                                    t1=t1_slab, gs=gs_slab)

                    # ---- slot matmuls for tile t ----
                    ps_S = psSpool.tile([128, 128], f32, tag="psS")
                    nmm = 0
                    for b in range(N_BANKS):
                        for j in range(caps[b]):
                            slot = b_off[b] + ti * caps[b] + j
                            k = cap_off[b] + j
                            col = (ti * S_t + k) * 128
                            nc.tensor.matmul(
                                ps_S[:], lhsT=gctx["G"][:, slot, :],
                                rhs=gctx["PTt"][:, col : col + 128],
                                start=(nmm == 0), stop=(nmm == S_t - 1))
                            nmm += 1

                # ---- finish previous tile: MM_B + sigmoid + t1 ----
                if pending is not None:
                    p_psY, p_segT, p_gctx, p_ti = pending
                    nc.tensor.matmul(p_psY[:], lhsT=p_segT[:], rhs=rhsB_sb[:],
                                     start=False, stop=True)
                    nc.scalar.activation(out=p_gctx["gs"][:, p_ti, :],
                                         in_=p_psY[:, D:], func=Act.Sigmoid)
                    nc.vector.tensor_tensor(out=p_gctx["t1"][:, p_ti, :],
                                            in0=p_psY[:, :D],
                                            in1=p_gctx["h_g"][:, p_ti, :],
                                            op=Alu.subtract)
                    if p_ti == TG - 1:
                        group_tail(p_gctx)
                    pending = None

                if t < TPC:
                    segT = segpool.tile([128, 128], bf16, tag="segT")
                    nc.scalar.copy(out=segT[:], in_=ps_S[:])
                    ps_Y = psYpool.tile([128, 2 * D], f32, tag="psY")
                    nc.tensor.matmul(ps_Y[:],
                                     lhsT=hT_sb[:, t * 128 : (t + 1) * 128],
                                     rhs=rhsA_sb[:], start=True, stop=False)
                    nc.tensor.matmul(ps_Y[:],
                                     lhsT=onesp_sb[:, t * 128 : (t + 1) * 128],
                                     rhs=biasAB_sb[:], start=False, stop=False)
                    pending = (ps_Y, segT, gctx, ti)
    nc.compile()
    return nc


def kernel(**inputs):
    from concourse.bass_utils import run_bass_kernel_spmd

    per_core, meta = _prep(**{k: np.asarray(v) for k, v in inputs.items()})
    idxcols = per_core[0]["idx_all"].shape[1]
    key = (tuple(meta["caps"]), meta["trivial_ln"], idxcols)
    if key not in _BASS_CACHE:
        _BASS_CACHE[key] = _build_bass(
            meta["caps"], meta["S_t"], meta["cap_off"], meta["trivial_ln"],
            idxcols)
    nc = _BASS_CACHE[key]

    in_maps = []
    for pc in per_core:
        m = dict(pc)
        if not meta["trivial_ln"]:
            m["ln_gamma"] = meta["ln_gamma"][None]
            m["ln_beta"] = meta["ln_beta"][None]
        in_maps.append(m)
    res = run_bass_kernel_spmd(nc, in_maps, core_ids=list(range(N_CORES)))
    outs = [res.results[c]["out"] for c in range(N_CORES)]
    return np.concatenate(outs, 0)[:N].astype(np.float32)
